# revision 5
# baseline (speedup 1.0000x reference)
"""Trainium2 Bass kernel for the batched 2D Kalman filter (nn_KalmanFilterWrapper).

Math
----
The reference runs, per trajectory, a Kalman filter over T=4096 steps with a
constant-velocity model.  The gain/covariance recursion (Riccati) is
data-independent, so the scan collapses to a linear time-varying recurrence

    x_t = A_t x_{t-1} + k_t z_t,        y_t = x_t[0]

with coefficients shared across the whole batch.  The 4-state filter decouples
into two identical 2-state (position, velocity) scalar filters — one per
coordinate — giving B*2 = 8192 independent scalar sequences.

The recurrence coefficients converge to steady state by t~135, and the steady
transition matrix has spectral radius 0.9315, so the filter's impulse response
g_d decays below 1e-6 by d=192.  Each aligned 128-step output chunk therefore
depends (to ~1e-5, vs a 2e-2 accuracy gate) only on the 256 measurements in
its own and the preceding 128-step input block:

    y[128*ci : 128*(ci+1)] = W_lo @ z_prev_block + W_hi @ z_this_block

where (W_lo, W_hi) are one shared Toeplitz pair built from g for all ci >= 2,
exact time-varying matrices for ci == 1, and a single exact lower-triangular
matrix for ci == 0 (which also folds in the x0 = [z_0, 0] initial condition).
All 32 chunks are INDEPENDENT matmuls — no serial carry chain at all.

Everything (measurements, weights, outputs) is bf16 on the wire; matmuls
accumulate in fp32 PSUM.  Host-side float64 weight construction + end-to-end
numpy simulation puts the total l2 relative error at ~3e-3 (truncation alone:
2.4e-5).

Sharding: data-parallel across 8 NeuronCores, 512 trajectories (1024 scalar
sequences) per core.  Layout on device is [time, sequence]; the host
transposes/casts in and out of the reference's [batch, time, 2] fp32 layout.
DMA is batched in 1 MiB transfers (4 x 128 time rows); input DMAs ride the
sync HWDGE ring, output DMAs the scalar ring so they never queue behind each
other.
"""

import numpy as np
import ml_dtypes

import concourse.bass as bass
import concourse.bacc as bacc
import concourse.mybir as mybir
from concourse.bass_utils import run_bass_kernel_spmd
from concourse.tile import TileContext

# Problem constants (hardcoded per harness contract).
B = 4096
T = 4096
DT = 1.0
PROCESS_VARIANCE = 1e-05
MEASUREMENT_VARIANCE = 0.1
INIT_ERROR = 1.0

N_CORES = 8
NCOLS = (B * 2) // N_CORES  # 1024 scalar sequences per core
CHUNK = 512                 # matmul free dim (one fp32 PSUM bank)
GROUP = 4                   # 128-row blocks per DMA transfer (1 MiB)
NBLK = T // 128             # 32 output chunks
NGRP = NBLK // GROUP        # 8 DMA groups
NSLOT = 5                   # weight matrices: W0, Wlo1, Whi1, WloS, WhiS

BF16 = mybir.dt.bfloat16
F32 = mybir.dt.float32
NPBF16 = ml_dtypes.bfloat16


def _precompute_lhsT():
    """Host-side Riccati + chunk weight matrices, float64 -> bf16.

    Returns [128, 5*128] bf16; slot s holds lhsT = W_s.T so that
    matmul(out, lhsT, z) computes out[t, n] = sum_k W_s[t, k] z[k, n].
    """
    F = np.array([[1.0, DT], [0.0, 1.0]], dtype=np.float64)
    I2 = np.eye(2, dtype=np.float64)
    P = INIT_ERROR * I2.copy()
    A = np.zeros((T, 2, 2), dtype=np.float64)
    k = np.zeros((T, 2), dtype=np.float64)
    for t in range(T):
        Pp = F @ P @ F.T + PROCESS_VARIANCE * I2
        s = Pp[0, 0] + MEASUREMENT_VARIANCE
        kt = Pp[:, 0] / s
        k[t] = kt
        KH = np.zeros((2, 2), dtype=np.float64)
        KH[:, 0] = kt
        P = (I2 - KH) @ Pp
        A[t] = (I2 - KH) @ F

    # Exact input->output operator over the first 256 steps.  Rc[:, j] is the
    # coefficient of measurement z_j in the current state; the initial state
    # is x_{-1} = [z_0, 0].
    W = np.zeros((256, 256), dtype=np.float64)
    Rc = np.zeros((2, 256), dtype=np.float64)
    Rc[0, 0] = 1.0
    for t in range(256):
        Rc = A[t] @ Rc
        Rc[:, t] += k[t]
        W[t] = Rc[0]

    # Steady-state impulse response g_d = [Ainf^d kinf][0].
    g = np.zeros(256, dtype=np.float64)
    vv = k[-1].copy()
    for d in range(256):
        g[d] = vv[0]
        vv = A[-1] @ vv
    m, kk = np.mgrid[0:128, 0:128]
    WloS = g[m + 128 - kk]
    WhiS = np.where(m >= kk, g[np.abs(m - kk)], 0.0)

    slots = [
        W[0:128, 0:128],      # chunk 0 (exact, incl. initial condition)
        W[128:256, 0:128],    # chunk 1 lo (exact transient)
        W[128:256, 128:256],  # chunk 1 hi
        WloS,                 # chunks 2..31 lo (steady Toeplitz)
        WhiS,                 # chunks 2..31 hi
    ]
    lhsT = np.zeros((128, NSLOT * 128), dtype=np.float64)
    for s, Ws in enumerate(slots):
        lhsT[:, s * 128:(s + 1) * 128] = Ws.T
    return np.ascontiguousarray(lhsT.astype(NPBF16))


def _build_nc():
    # Device layout (host pre-swizzled): z/v are [128 partitions, 32 blocks x
    # 1024 seqs]; partition p of block bi holds time step 128*bi + p.  Every
    # DMA is then fully contiguous per partition line.
    nc = bacc.Bacc()
    z = nc.dram_tensor("z", [128, NBLK * NCOLS], BF16, kind="ExternalInput")
    u = nc.dram_tensor("u", [128, NSLOT * 128], BF16, kind="ExternalInput")
    v = nc.dram_tensor("v", [128, NBLK * NCOLS], BF16, kind="ExternalOutput")

    nchunks = NCOLS // CHUNK

    with TileContext(nc) as tc:
        with (
            tc.tile_pool(name="consts", bufs=1) as cpool,
            tc.tile_pool(name="zpool", bufs=6) as zpool,
            tc.tile_pool(name="vpool", bufs=4) as vpool,
            tc.tile_pool(name="psum", bufs=8, space="PSUM") as ppool,
        ):
            u_tile = cpool.tile([128, NSLOT * 128], BF16)
            nc.sync.dma_start(u_tile[:, :], u[:, :])

            # 256 KiB per-block input loads on the sync HWDGE ring.
            ztiles = []
            for bi in range(NBLK):
                zp = zpool.tile([128, NCOLS], BF16, name=f"z{bi}", tag="zp")
                nc.sync.dma_start(zp[:, :], z[:, bass.ds(bi * NCOLS, NCOLS)])
                ztiles.append(zp)

            for ci in range(NBLK):
                vout = vpool.tile([128, NCOLS], BF16, name=f"v{ci}", tag="vout")
                zhi = ztiles[ci]
                ps = [
                    ppool.tile([128, CHUNK], F32, name=f"ps{cc}", tag="ps")
                    for cc in range(nchunks)
                ]
                if ci == 0:
                    for cc in range(nchunks):
                        cols = bass.ds(cc * CHUNK, CHUNK)
                        nc.tensor.matmul(
                            ps[cc][:, :],
                            u_tile[:, bass.ds(0, 128)],
                            zhi[:, cols],
                            start=True,
                            stop=True,
                        )
                else:
                    zlo = ztiles[ci - 1]
                    lo_slot, hi_slot = (1, 2) if ci == 1 else (3, 4)
                    # lo over both col-chunks, then hi: consecutive matmuls
                    # share the stationary operand.
                    for cc in range(nchunks):
                        cols = bass.ds(cc * CHUNK, CHUNK)
                        nc.tensor.matmul(
                            ps[cc][:, :],
                            u_tile[:, bass.ds(lo_slot * 128, 128)],
                            zlo[:, cols],
                            start=True,
                            stop=False,
                        )
                    for cc in range(nchunks):
                        cols = bass.ds(cc * CHUNK, CHUNK)
                        nc.tensor.matmul(
                            ps[cc][:, :],
                            u_tile[:, bass.ds(hi_slot * 128, 128)],
                            zhi[:, cols],
                            start=False,
                            stop=True,
                        )
                # split PSUM evictions across ACT and DVE
                for cc in range(nchunks):
                    cols = bass.ds(cc * CHUNK, CHUNK)
                    if (ci + cc) % 2 == 0:
                        nc.scalar.copy(vout[:, cols], ps[cc][:, :])
                    else:
                        nc.vector.tensor_copy(vout[:, cols], ps[cc][:, :])
                # 256 KiB output store on the scalar HWDGE ring (input loads
                # ride the sync ring, so the two never queue behind each
                # other).
                nc.scalar.dma_start(v[:, bass.ds(ci * NCOLS, NCOLS)], vout[:, :])
    nc.finalize()  # Bacc.compile(): splits multi-waits, allocates registers
    return nc


_CACHE = {}


def _run(x_seq: np.ndarray, trace: bool = False):
    if "nc" not in _CACHE:
        _CACHE["nc"] = _build_nc()
        _CACHE["u"] = _precompute_lhsT()
    nc = _CACHE["nc"]
    u_all = _CACHE["u"]

    x = np.asarray(x_seq, dtype=np.float32)
    assert x.shape == (B, T, 2), x.shape

    # [B, T, 2] -> [T, B*2] bf16; column n = 2*b + c.  Then swizzle each
    # core's [T, NCOLS] shard into the device layout [128, NBLK*NCOLS]
    # (partition = t % 128, block-major free dim) so DMAs are contiguous.
    zt = np.ascontiguousarray(x.transpose(1, 0, 2).reshape(T, B * 2)).astype(NPBF16)

    in_maps = []
    for i in range(N_CORES):
        zc = (
            zt[:, i * NCOLS:(i + 1) * NCOLS]
            .reshape(NBLK, 128, NCOLS)
            .transpose(1, 0, 2)
            .reshape(128, NBLK * NCOLS)
        )
        in_maps.append({"z": np.ascontiguousarray(zc), "u": u_all})
    res = run_bass_kernel_spmd(nc, in_maps, core_ids=list(range(N_CORES)), trace=trace)

    # inverse swizzle: [128, NBLK*NCOLS] -> [T, NCOLS], then concat cores
    vt = np.concatenate(
        [
            r["v"].reshape(128, NBLK, NCOLS).transpose(1, 0, 2).reshape(T, NCOLS)
            for r in res.results
        ],
        axis=1,
    )  # [T, B*2] bf16
    out = np.ascontiguousarray(
        vt.astype(np.float32).reshape(T, B, 2).transpose(1, 0, 2)
    )
    return out, res


def kernel(x_seq: np.ndarray) -> np.ndarray:
    out, _ = _run(x_seq, trace=False)
    return out
